# revision 11
# baseline (speedup 1.0000x reference)
"""NextVLAD Trainium2 kernel v2 — 8-way data-parallel, gk-partition fc_gk layout.

Per-core dataflow (M=512 tokens, N=1024, EN=2048, G=8, K=128, D=256):
  xp[c][n2,2,m]  <- fp8 x chunk pairs (DoubleRow contraction pairs over n)
  xsq            = x*x (gpsimd)   ; ss[*,m] = ones-matmul partition reduce
  inv[*,m]       = 1/sqrt(256*ss) (scalar Sqrt + DVE reciprocal_approx_fast)
  pair e2: y_ps[s][e,m] = (W1 e2-col-slice).T X  (DR fp8, 8 MMs)
           ybp[e2] = 32*y_ps*inv (fp8, DVE STT)
           yT[m][:, e] = PE-transpose(ybp) -> bf16 copies  (no b_inp: folded into corr MM)
           lg_ps[g][gk,m] += (w2 pair).T ybp[e2]  (DR fp8; g0..1 during phase1, rest after)
  gate: z[g,m] = (WgWinp).T X * inv ; sg = 0.5+0.5*tanh(0.5 z + 0.5 bg') (exp-set table)
        sgb[g][128,m] = partition_broadcast(sg[g])  (gpsimd)
  per g: ex[gk,m] = Exp(lg/256) bf16, accum_out -> se ; ise = 1/se (DVE tiny)
         wf[g] = (ex*ise)*sgb bf16, accum_out -> Sg[:,g]
         wfT[m,k] = PE-transpose(wf) -> bf16 ; einsum vd[k,d] += wfT_gm.T @ yT[m, gD]
  corr: SgT = transpose(Sg) ; vd[:,0:256] += SgT.T @ (32*binp[8,256]) (stop)
        vd[:,256] = SgT.T @ 32-ones  (= 32*S[k])
  vlad = vd[:,0:256] + (-cent)*vd[:,256] ; out = vlad * rsqrt(128*sum vlad^2)
All terms carry a global 32x scale that the final l2norm cancels.
"""
import os
import numpy as np

N = 1024
EN = 2048
G = 8
KC = 128
D = 256
M = 512
NE2 = 8           # e2 pairs (256-contraction) in EN
MT = 4            # m-tiles

_cache = {}

# feature flags (HW-suspect ops; flip back on once exonerated)
USE_GPSIMD_XSQ = False    # gpsimd/vector fp8 tensor_mul for x*x
USE_RECIP_APPROX = False  # vector.reciprocal_approx_fast for 1/nrm
USE_ACCUM_OUT = False     # accum_out on Exp / wf STT
USE_TTR = False           # tensor_tensor_reduce for final l2norm


def _build_nc():
    import concourse.bacc as bacc
    import concourse.tile as tile
    from concourse import mybir

    f32 = mybir.dt.float32
    bf16 = mybir.dt.bfloat16
    fp8 = mybir.dt.float8e4
    Alu = mybir.AluOpType
    Act = mybir.ActivationFunctionType
    DR = mybir.MatmulPerfMode.DoubleRow

    nc = bacc.Bacc("TRN2", target_bir_lowering=False)
    x_d = nc.dram_tensor("x", [N, M], fp8, kind="ExternalInput")
    w1_d = nc.dram_tensor("w1", [N, EN + G], fp8, kind="ExternalInput")
    w2_d = nc.dram_tensor("w2", [EN, G * KC], fp8, kind="ExternalInput")
    cbf_d = nc.dram_tensor("cbf", [128, 1280], bf16, kind="ExternalInput")  # ones|eye|onehot
    cf8_d = nc.dram_tensor("cf8", [128, 128], fp8, kind="ExternalInput")   # eye
    cf32_d = nc.dram_tensor("cf32", [128, 384], f32, kind="ExternalInput")  # -cent|eye
    bp_d = nc.dram_tensor("bp", [G, 260], f32, kind="ExternalInput")
    out_d = nc.dram_tensor("out", [KC, D], f32, kind="ExternalOutput")

    with tile.TileContext(nc) as tc:
        with tc.tile_pool(name="sb", bufs=1) as sb:
            # ---------------- DMAs ----------------
            xp = [sb.tile([128, 2 * M], fp8, name=f"xp{c}") for c in range(4)]
            xpv = [t.rearrange("p (s m) -> p s m", m=M) for t in xp]
            nc.gpsimd.dma_start(
                out=xpv[0],
                in_=x_d[0:256, :].rearrange("(s p) m -> p s m", p=128))
            cbf_t = sb.tile([128, 1280], bf16)
            nc.gpsimd.dma_start(out=cbf_t[:], in_=cbf_d[:])
            ones_bf = cbf_t[:, 0:128]
            identbf = cbf_t[:, 128:256]
            onehot = cbf_t[:, 256:1280]
            cf8_t = sb.tile([128, 128], fp8)
            nc.gpsimd.dma_start(out=cf8_t[:], in_=cf8_d[:])
            cf32_t = sb.tile([128, 384], f32)
            nc.scalar.dma_start(out=cf32_t[:], in_=cf32_d[:])
            centn_t = cf32_t[:, 0:256]
            identf32 = cf32_t[:, 256:384]
            bp_t = sb.tile([G, 260], f32)
            nc.scalar.dma_start(out=bp_t[:], in_=bp_d[:])

            for c in range(1, 4):
                nc.gpsimd.dma_start(
                    out=xpv[c],
                    in_=x_d[c * 256:(c + 1) * 256, :].rearrange(
                        "(s p) m -> p s m", p=128))
            # w1 column-slices: one DMA per e2 pair -> [128, 4(c), 2(s), 256]
            w1s = [sb.tile([128, 2048], fp8, name=f"w1s{e2}") for e2 in range(NE2)]
            w1sv = [t.rearrange("p (c s e) -> p c s e", c=4, s=2) for t in w1s]
            for e2 in range(NE2):
                src = w1_d[:, e2 * 256:(e2 + 1) * 256].rearrange(
                    "(c s p) e -> p c s e", p=128, s=2)
                nc.sync.dma_start(out=w1sv[e2], in_=src)
            # pad gate e-dim to 16 so the DoubleRow Ko step is 16 bytes
            wg = sb.tile([128, 128], fp8, name="wg")
            wgv = wg.rearrange("p (c s e) -> p c s e", c=4, s=2, e=16)
            nc.sync.dma_start(
                out=wgv[:, :, :, 0:G],
                in_=w1_d[:, EN:EN + G].rearrange("(c s p) e -> p c s e", p=128, s=2))
            w2p = [sb.tile([128, 2 * G * KC], fp8, name=f"w2p{e2}") for e2 in range(NE2)]
            w2v = [t.rearrange("p (s j) -> p s j", j=G * KC) for t in w2p]
            for e2 in range(NE2):
                nc.sync.dma_start(
                    out=w2v[e2],
                    in_=w2_d[e2 * 256:(e2 + 1) * 256, :].rearrange(
                        "(s p) j -> p s j", p=128))

            # ---------------- persistent SBUF ----------------
            xsq = [sb.tile([128, 2 * M], bf16, name=f"xsq{c}") for c in range(4)]
            xsqv = [t.rearrange("p (s m) -> p s m", m=M) for t in xsq]
            inv_t = sb.tile([128, M], f32)
            nrm_t = sb.tile([128, M], f32)
            ybp = [sb.tile([128, 2 * M], fp8, name=f"ybp{e2}") for e2 in range(NE2)]
            ybv = [t.rearrange("p (s m) -> p s m", m=M) for t in ybp]
            yT = [sb.tile([128, EN], bf16, name=f"yT{m}") for m in range(MT)]
            sgs_t = sb.tile([G, M], f32)
            sg_t = sb.tile([G, M], bf16)
            sgb = [sb.tile([128, M], bf16, name=f"sgb{g}") for g in range(G)]
            ex_t = [sb.tile([128, M], bf16, name=f"ex{g}") for g in range(G)]
            se_t = sb.tile([128, G], f32)
            ise_t = sb.tile([128, G], f32)
            wf_t = [sb.tile([128, M], bf16, name=f"wf{g}") for g in range(G)]
            wfT_t = [sb.tile([128, M], bf16, name=f"wfT{g}") for g in range(G)]
            sgs_acc = sb.tile([128, G], f32, name="sgacc")
            sgT_sb = sb.tile([G, 128], f32, name="sgT")
            vlad_t = sb.tile([128, D], f32)
            sq_t = sb.tile([128, D], f32)
            ss2_t = sb.tile([128, 1], f32)
            nr2_t = sb.tile([128, 1], f32)
            r1_t = sb.tile([128, 1], f32)
            out_t = sb.tile([128, D], f32)

            # scalar engine: preload sqrt table (dummy), real work later
            nc.scalar.activation(nrm_t[0:1, 0:1], bp_t[0:1, 256:257], Act.Sqrt)

            # xsq on the DVE (idle early; keeps the scalar queue clear for the
            # sqrt-table preload + inv sqrt)
            for c in range(4):
                if USE_GPSIMD_XSQ:
                    eng = nc.gpsimd if c < 2 else nc.vector
                    eng.tensor_mul(xsq[c][:], xp[c][:], xp[c][:])
                else:
                    nc.vector.tensor_mul(xsq[c][:], xp[c][:], xp[c][:])

            with tc.tile_pool(name="lgp", bufs=1, space="PSUM") as lgp:
                lg_ps = [None] * G

                def lg_tile(g):
                    lg_ps[g] = lgp.tile([128, M], f32, name=f"lg{g}", tag="lg", bufs=3)

                def lg_chain(g, e2):
                    nc.tensor.matmul(lg_ps[g][:], w2v[e2][:, :, g * KC:(g + 1) * KC],
                                     ybv[e2], start=(e2 == 0), stop=(e2 == NE2 - 1),
                                     perf_mode=DR)

                with tc.tile_pool(name="ps1", bufs=1, space="PSUM") as ps1:
                    # sum of squares -> inv
                    ss_ps = ps1.tile([128, M], f32, name="ss", tag="ssg", bufs=1)
                    for c in range(4):
                        for s in range(2):
                            nc.tensor.matmul(ss_ps[:], ones_bf, xsqv[c][:, s, :],
                                             start=(c == 0 and s == 0),
                                             stop=(c == 3 and s == 1))
                    nc.scalar.activation(nrm_t[:], ss_ps[:], Act.Sqrt, scale=256.0)
                    if USE_RECIP_APPROX:
                        nc.vector.reciprocal_approx_fast(out=inv_t[:], in_=nrm_t[:])
                    else:
                        nc.vector.reciprocal(inv_t[:], nrm_t[:])
                    # preload exp table behind the DMA/matmul wait
                    nc.scalar.activation(sgs_t[0:1, 0:1], bp_t[0:1, 256:257], Act.Exp)

                    def pair(e2):
                        yps = [ps1.tile([128, M], f32, name=f"y{e2}_{s}", tag="mm",
                                        bufs=4) for s in range(2)]
                        for c in range(4):
                            for s in range(2):
                                nc.tensor.matmul(
                                    yps[s][:], w1sv[e2][:, c, :, s * 128:(s + 1) * 128],
                                    xpv[c], start=(c == 0), stop=(c == 3),
                                    perf_mode=DR)
                        for s in range(2):
                            nc.vector.scalar_tensor_tensor(
                                out=ybv[e2][:, s, :], in0=yps[s][:], scalar=32.0,
                                in1=inv_t[:], op0=Alu.mult, op1=Alu.mult)

                    def transposes(e2):
                        # 8 fp8 transposes (2 e-tiles x 4 m) packed in ONE 2KB bank
                        # (fp8 transpose requires output element step of 2)
                        tp = ps1.tile([128, 2048], fp8, name=f"t{e2}",
                                      tag="mm", bufs=4)
                        tpv = tp.rearrange("p (k e two) -> p k e two", k=8, two=2)
                        for m in range(MT):
                            for s in range(2):
                                k = m * 2 + s
                                nc.tensor.transpose(
                                    tpv[:, k, :, 0],
                                    ybv[e2][:, s, m * 128:(m + 1) * 128], cf8_t)
                        for m in range(MT):
                            # scalar engine (idle in phase 1) does the psum->bf16
                            # copies; DVE stays free for ybp STTs
                            nc.scalar.activation(
                                yT[m][:, e2 * 256:(e2 + 1) * 256].rearrange(
                                    "p (s e) -> p s e", s=2),
                                tpv[:, 2 * m:2 * m + 2, :, 0], Act.Copy)

                    # emission: pair0, pair1, gate, then lag-1 T/lg with pairs
                    pair(0)
                    pair(1)
                    gate_ps = ps1.tile([G, M], f32, name="gate", tag="ssg", bufs=1)
                    for c in range(4):
                        nc.tensor.matmul(gate_ps[:], wgv[:, c, :, 0:G], xpv[c],
                                         start=(c == 0), stop=(c == 3), perf_mode=DR)
                    nc.vector.tensor_mul(sgs_t[:], gate_ps[:], inv_t[0:G, :])
                    nc.scalar.activation(sgs_t[:], sgs_t[:], Act.Tanh,
                                         bias=bp_t[:, 258:259], scale=0.5)
                    nc.vector.tensor_scalar(out=sg_t[:], in0=sgs_t[:], scalar1=0.5,
                                            scalar2=0.5, op0=Alu.mult, op1=Alu.add)
                    lg_tile(0)
                    lg_tile(1)
                    for e2 in range(NE2):
                        if e2 + 2 < NE2:
                            pair(e2 + 2)
                        transposes(e2)
                        lg_chain(0, e2)
                        lg_chain(1, e2)

                # ---------------- phase 2/3 ----------------
                with tc.tile_pool(name="ps3", bufs=1, space="PSUM") as ps3:
                    vd_ps = ps3.tile([128, 512], f32, name="vd", tag="vd", bufs=1)

                    # broadcast sg rows to 128 partitions via one-hot matmul
                    for g in range(G):
                        sgb_ps = ps3.tile([128, M], f32, name=f"sgb{g}", tag="p3",
                                          bufs=3)
                        nc.tensor.matmul(
                            sgb_ps[:], onehot[0:G, g * 128:(g + 1) * 128],
                            sg_t[:], start=True, stop=True)
                        nc.vector.tensor_copy(sgb[g][:], sgb_ps[:])

                    def softwf(g):
                        if USE_ACCUM_OUT:
                            nc.scalar.activation(ex_t[g][:], lg_ps[g][:], Act.Exp,
                                                 scale=1.0 / 256.0,
                                                 accum_out=se_t[:, g:g + 1])
                        else:
                            nc.scalar.activation(ex_t[g][:], lg_ps[g][:], Act.Exp,
                                                 scale=1.0 / 256.0)
                            nc.vector.reduce_sum(out=se_t[:, g:g + 1],
                                                 in_=ex_t[g][:],
                                                 axis=mybir.AxisListType.X)
                        nc.vector.reciprocal(ise_t[:, g:g + 1], se_t[:, g:g + 1])
                        if USE_ACCUM_OUT:
                            nc.vector.scalar_tensor_tensor(
                                out=wf_t[g][:], in0=ex_t[g][:],
                                scalar=ise_t[:, g:g + 1],
                                in1=sgb[g][:], op0=Alu.mult, op1=Alu.mult,
                                accum_out=sgs_acc[:, g:g + 1])
                        else:
                            nc.vector.scalar_tensor_tensor(
                                out=wf_t[g][:], in0=ex_t[g][:],
                                scalar=ise_t[:, g:g + 1],
                                in1=sgb[g][:], op0=Alu.mult, op1=Alu.mult)
                            nc.vector.reduce_sum(out=sgs_acc[:, g:g + 1],
                                                 in_=wf_t[g][:],
                                                 axis=mybir.AxisListType.X)

                    def wfT_einsum(g):
                        tp = ps3.tile([128, 1024], bf16, name=f"wt{g}", tag="p3",
                                      bufs=3)
                        for m in range(MT):
                            nc.tensor.transpose(tp[:, m * 128:(m + 1) * 128],
                                                wf_t[g][:, m * 128:(m + 1) * 128],
                                                identbf)
                        nc.vector.tensor_copy(wfT_t[g][:], tp[:, 0:M])
                        for m in range(MT):
                            nc.tensor.matmul(
                                vd_ps[:, 0:256],
                                wfT_t[g][:, m * 128:(m + 1) * 128],
                                yT[m][:, g * 256:(g + 1) * 256],
                                start=(g == 0 and m == 0), stop=False)

                    for gp in range(1, 4):
                        g0, g1 = 2 * gp, 2 * gp + 1
                        pv0, pv1 = 2 * (gp - 1), 2 * (gp - 1) + 1
                        softwf(pv0)
                        softwf(pv1)
                        lg_tile(g0)
                        for e2 in range(NE2):
                            lg_chain(g0, e2)
                        wfT_einsum(pv0)
                        lg_tile(g1)
                        for e2 in range(NE2):
                            lg_chain(g1, e2)
                        wfT_einsum(pv1)
                    for g in (6, 7):
                        softwf(g)
                        wfT_einsum(g)
                    # swap back to sqrt table behind the einsum tail
                    nc.scalar.activation(nrm_t[0:1, 0:1], bp_t[0:1, 256:257], Act.Sqrt)

                    # correction: SgT = transpose(Sg); vd += SgT.T @ [32*binp | 32]
                    sgT_ps = ps3.tile([128, 512], f32, name="sgT", tag="p3", bufs=3)
                    nc.tensor.transpose(sgT_ps[0:G, 0:128], sgs_acc[:], identf32)
                    nc.vector.tensor_copy(sgT_sb[:], sgT_ps[0:G, 0:128])
                    nc.tensor.matmul(vd_ps[:, 0:256], sgT_sb[:], bp_t[:, 0:256],
                                     start=False, stop=True)
                    nc.tensor.matmul(vd_ps[:, 256:257], sgT_sb[:], bp_t[:, 256:257],
                                     start=True, stop=True)

                    nc.vector.scalar_tensor_tensor(
                        out=vlad_t[:], in0=centn_t[:], scalar=vd_ps[:, 256:257],
                        in1=vd_ps[:, 0:256], op0=Alu.mult, op1=Alu.add)
                    if USE_TTR:
                        nc.vector.tensor_tensor_reduce(
                            out=sq_t[:], in0=vlad_t[:], in1=vlad_t[:], scale=1.0,
                            scalar=0.0, op0=Alu.mult, op1=Alu.add,
                            accum_out=ss2_t[:])
                    else:
                        nc.vector.tensor_mul(sq_t[:], vlad_t[:], vlad_t[:])
                        nc.vector.reduce_sum(out=ss2_t[:], in_=sq_t[:],
                                             axis=mybir.AxisListType.X)
                    nc.scalar.activation(nr2_t[:], ss2_t[:], Act.Sqrt, scale=128.0)
                    nc.vector.reciprocal(r1_t[:], nr2_t[:])
                    nc.vector.tensor_scalar_mul(out_t[:], vlad_t[:], r1_t[:])
                    nc.sync.dma_start(out=out_d[:], in_=out_t[:])

    nc.compile()
    return nc


def _get_nc():
    if "nc" not in _cache:
        _cache["nc"] = _build_nc()
    return _cache["nc"]


def host_inputs(x, W_inp, b_inp, W_g, b_g, W_gk, b_gk, centroids):
    import ml_dtypes as mld

    x = np.asarray(x, dtype=np.float32)
    X = x.reshape(8, 8, N, 64).transpose(0, 2, 1, 3).reshape(8, N, M)
    WgT = ((np.asarray(W_g, np.float64) @ np.asarray(W_inp, np.float64)).T
           ).astype(np.float32)
    W1 = np.ascontiguousarray(
        (np.concatenate([np.asarray(W_inp, np.float32).T, WgT],
                        axis=1) * 16.0).astype(mld.float8_e4m3))
    W2 = np.ascontiguousarray(
        (np.asarray(W_gk, np.float32).T * 8.0).astype(mld.float8_e4m3))
    bg = (np.asarray(b_g, np.float64)
          + np.asarray(W_g, np.float64) @ np.asarray(b_inp, np.float64)
          ).astype(np.float32)
    oh = np.zeros((128, 1024), np.float32)
    for g in range(G):
        oh[g, g * 128:(g + 1) * 128] = 1.0
    cbf = np.concatenate([np.ones((128, 128), np.float32),
                          np.eye(128, dtype=np.float32), oh],
                         axis=1).astype(mld.bfloat16)
    cf8 = np.eye(128, dtype=np.float32).astype(mld.float8_e4m3)
    cf32 = np.concatenate([-np.asarray(centroids, np.float32),
                           np.eye(128, dtype=np.float32)], axis=1)
    cf32 = np.ascontiguousarray(cf32)
    bp = np.zeros((G, 260), np.float32)
    bp[:, 0:256] = np.asarray(b_inp, np.float32).reshape(G, D) * 32.0
    bp[:, 256] = 32.0
    bp[:, 258] = 0.5 * bg
    common = {"w1": W1, "w2": W2, "cbf": np.ascontiguousarray(cbf),
              "cf8": np.ascontiguousarray(cf8), "cf32": cf32, "bp": bp}
    in_maps = []
    for b in range(8):
        m = dict(common)
        m["x"] = np.ascontiguousarray((X[b] * 8.0).astype(mld.float8_e4m3))
        in_maps.append(m)
    return in_maps


def kernel(x, W_inp, b_inp, W_g, b_g, W_gk, b_gk, centroids):
    from concourse.bass_utils import run_bass_kernel_spmd

    nc = _get_nc()
    in_maps = host_inputs(x, W_inp, b_inp, W_g, b_g, W_gk, b_gk, centroids)
    trace = os.environ.get("KERNEL_TRACE") == "1"
    r = run_bass_kernel_spmd(nc, in_maps, core_ids=list(range(8)), trace=trace)
    _cache["last_results"] = r
    return np.stack([r.results[b]["out"].reshape(KC * D)
                     for b in range(8)]).astype(np.float32)


# revision 13
# speedup vs baseline: 1.0112x; 1.0112x over previous
"""NextVLAD Trainium2 kernel v2 — 8-way data-parallel, gk-partition fc_gk layout.

Per-core dataflow (M=512 tokens, N=1024, EN=2048, G=8, K=128, D=256):
  xp[c][n2,2,m]  <- fp8 x chunk pairs (DoubleRow contraction pairs over n)
  xsq            = x*x (gpsimd)   ; ss[*,m] = ones-matmul partition reduce
  inv[*,m]       = 1/sqrt(256*ss) (scalar Sqrt + DVE reciprocal_approx_fast)
  pair e2: y_ps[s][e,m] = (W1 e2-col-slice).T X  (DR fp8, 8 MMs)
           ybp[e2] = 32*y_ps*inv (fp8, DVE STT)
           yT[m][:, e] = PE-transpose(ybp) -> bf16 copies  (no b_inp: folded into corr MM)
           lg_ps[g][gk,m] += (w2 pair).T ybp[e2]  (DR fp8; g0..1 during phase1, rest after)
  gate: z[g,m] = (WgWinp).T X * inv ; sg = 0.5+0.5*tanh(0.5 z + 0.5 bg') (exp-set table)
        sgb[g][128,m] = partition_broadcast(sg[g])  (gpsimd)
  per g: ex[gk,m] = Exp(lg/256) bf16, accum_out -> se ; ise = 1/se (DVE tiny)
         wf[g] = (ex*ise)*sgb bf16, accum_out -> Sg[:,g]
         wfT[m,k] = PE-transpose(wf) -> bf16 ; einsum vd[k,d] += wfT_gm.T @ yT[m, gD]
  corr: SgT = transpose(Sg) ; vd[:,0:256] += SgT.T @ (32*binp[8,256]) (stop)
        vd[:,256] = SgT.T @ 32-ones  (= 32*S[k])
  vlad = vd[:,0:256] + (-cent)*vd[:,256] ; out = vlad * rsqrt(128*sum vlad^2)
All terms carry a global 32x scale that the final l2norm cancels.
"""
import os
import numpy as np

N = 1024
EN = 2048
G = 8
KC = 128
D = 256
M = 512
NE2 = 8           # e2 pairs (256-contraction) in EN
MT = 4            # m-tiles

_cache = {}

# feature flags (HW-suspect ops; flip back on once exonerated)
USE_GPSIMD_XSQ = False    # gpsimd/vector fp8 tensor_mul for x*x
USE_RECIP_APPROX = False  # vector.reciprocal_approx_fast for 1/nrm
USE_ACCUM_OUT = False     # accum_out on Exp / wf STT
USE_TTR = False           # tensor_tensor_reduce for final l2norm


def _build_nc():
    import concourse.bacc as bacc
    import concourse.tile as tile
    from concourse import mybir

    f32 = mybir.dt.float32
    bf16 = mybir.dt.bfloat16
    fp8 = mybir.dt.float8e4
    Alu = mybir.AluOpType
    Act = mybir.ActivationFunctionType
    DR = mybir.MatmulPerfMode.DoubleRow

    nc = bacc.Bacc("TRN2", target_bir_lowering=False)
    x_d = nc.dram_tensor("x", [N, M], fp8, kind="ExternalInput")
    w1_d = nc.dram_tensor("w1", [N, EN + G], fp8, kind="ExternalInput")
    w2_d = nc.dram_tensor("w2", [EN, G * KC], fp8, kind="ExternalInput")
    cbf_d = nc.dram_tensor("cbf", [128, 1280], bf16, kind="ExternalInput")  # ones|eye|onehot
    cf8_d = nc.dram_tensor("cf8", [128, 128], fp8, kind="ExternalInput")   # eye
    cf32_d = nc.dram_tensor("cf32", [128, 384], f32, kind="ExternalInput")  # -cent|eye
    bp_d = nc.dram_tensor("bp", [G, 260], f32, kind="ExternalInput")
    out_d = nc.dram_tensor("out", [KC, D], f32, kind="ExternalOutput")

    with tile.TileContext(nc) as tc:
        with tc.tile_pool(name="sb", bufs=1) as sb:
            # ---------------- DMAs ----------------
            xp = [sb.tile([128, 2 * M], fp8, name=f"xp{c}") for c in range(4)]
            xpv = [t.rearrange("p (s m) -> p s m", m=M) for t in xp]
            for c in range(4):
                nc.gpsimd.dma_start(
                    out=xpv[c],
                    in_=x_d[c * 256:(c + 1) * 256, :].rearrange(
                        "(s p) m -> p s m", p=128))
            cbf_t = sb.tile([128, 1280], bf16)
            nc.gpsimd.dma_start(out=cbf_t[:], in_=cbf_d[:])
            ones_bf = cbf_t[:, 0:128]
            identbf = cbf_t[:, 128:256]
            onehot = cbf_t[:, 256:1280]
            cf8_t = sb.tile([128, 128], fp8)
            nc.gpsimd.dma_start(out=cf8_t[:], in_=cf8_d[:])
            cf32_t = sb.tile([128, 384], f32)
            nc.scalar.dma_start(out=cf32_t[:], in_=cf32_d[:])
            centn_t = cf32_t[:, 0:256]
            identf32 = cf32_t[:, 256:384]
            bp_t = sb.tile([G, 260], f32)
            nc.scalar.dma_start(out=bp_t[:], in_=bp_d[:])
            # w1 column-slices: one DMA per e2 pair -> [128, 4(c), 2(s), 256]
            w1s = [sb.tile([128, 2048], fp8, name=f"w1s{e2}") for e2 in range(NE2)]
            w1sv = [t.rearrange("p (c s e) -> p c s e", c=4, s=2) for t in w1s]
            for e2 in range(NE2):
                src = w1_d[:, e2 * 256:(e2 + 1) * 256].rearrange(
                    "(c s p) e -> p c s e", p=128, s=2)
                nc.sync.dma_start(out=w1sv[e2], in_=src)
            # pad gate e-dim to 16 so the DoubleRow Ko step is 16 bytes
            wg = sb.tile([128, 128], fp8, name="wg")
            wgv = wg.rearrange("p (c s e) -> p c s e", c=4, s=2, e=16)
            nc.sync.dma_start(
                out=wgv[:, :, :, 0:G],
                in_=w1_d[:, EN:EN + G].rearrange("(c s p) e -> p c s e", p=128, s=2))
            w2p = [sb.tile([128, 2 * G * KC], fp8, name=f"w2p{e2}") for e2 in range(NE2)]
            w2v = [t.rearrange("p (s j) -> p s j", j=G * KC) for t in w2p]
            for e2 in range(NE2):
                nc.sync.dma_start(
                    out=w2v[e2],
                    in_=w2_d[e2 * 256:(e2 + 1) * 256, :].rearrange(
                        "(s p) j -> p s j", p=128))

            # ---------------- persistent SBUF ----------------
            xsq = [sb.tile([128, 2 * M], bf16, name=f"xsq{c}") for c in range(4)]
            xsqv = [t.rearrange("p (s m) -> p s m", m=M) for t in xsq]
            inv_t = sb.tile([128, M], f32)
            nrm_t = sb.tile([128, M], f32)
            ybp = [sb.tile([128, 2 * M], fp8, name=f"ybp{e2}") for e2 in range(NE2)]
            ybv = [t.rearrange("p (s m) -> p s m", m=M) for t in ybp]
            yT = [sb.tile([128, EN], bf16, name=f"yT{m}") for m in range(MT)]
            sgs_t = sb.tile([G, M], f32)
            sg_t = sb.tile([G, M], bf16)
            sgb = [sb.tile([128, M], bf16, name=f"sgb{g}") for g in range(G)]
            ex_t = [sb.tile([128, M], bf16, name=f"ex{g}") for g in range(G)]
            se_t = sb.tile([128, G], f32)
            ise_t = sb.tile([128, G], f32)
            wf_t = [sb.tile([128, M], bf16, name=f"wf{g}") for g in range(G)]
            wfT_t = [sb.tile([128, M], bf16, name=f"wfT{g}") for g in range(G)]
            sgs_acc = sb.tile([128, G], f32, name="sgacc")
            sgT_sb = sb.tile([G, 128], f32, name="sgT")
            vlad_t = sb.tile([128, D], f32)
            sq_t = sb.tile([128, D], f32)
            ss2_t = sb.tile([128, 1], f32)
            nr2_t = sb.tile([128, 1], f32)
            r1_t = sb.tile([128, 1], f32)
            out_t = sb.tile([128, D], f32)

            # scalar engine: preload sqrt table (dummy), real work later
            nc.scalar.activation(nrm_t[0:1, 0:1], bp_t[0:1, 256:257], Act.Sqrt)

            # xsq on the DVE (idle early; keeps the scalar queue clear for the
            # sqrt-table preload + inv sqrt)
            for c in range(4):
                if USE_GPSIMD_XSQ:
                    eng = nc.gpsimd if c < 2 else nc.vector
                    eng.tensor_mul(xsq[c][:], xp[c][:], xp[c][:])
                else:
                    nc.vector.tensor_mul(xsq[c][:], xp[c][:], xp[c][:])

            with tc.tile_pool(name="lgp", bufs=1, space="PSUM") as lgp:
                lg_ps = [None] * G

                def lg_tile(g):
                    lg_ps[g] = lgp.tile([128, M], f32, name=f"lg{g}", tag="lg", bufs=3)

                def lg_chain(g, e2):
                    nc.tensor.matmul(lg_ps[g][:], w2v[e2][:, :, g * KC:(g + 1) * KC],
                                     ybv[e2], start=(e2 == 0), stop=(e2 == NE2 - 1),
                                     perf_mode=DR)

                with tc.tile_pool(name="ps1", bufs=1, space="PSUM") as ps1:
                    # HAM warmup: ~3.5us of junk matmuls on the first x chunk so
                    # the PE clock-gate is at 8/8 when real chains start
                    warm_ps = ps1.tile([128, M], f32, name="warm", tag="mm", bufs=4)
                    for w in range(12):
                        nc.tensor.matmul(warm_ps[:], cf8_t[:], xp[0][:, 0:M],
                                         start=True, stop=True)
                    # sum of squares -> inv
                    ss_ps = ps1.tile([128, M], f32, name="ss", tag="ssg", bufs=1)
                    for c in range(4):
                        for s in range(2):
                            nc.tensor.matmul(ss_ps[:], ones_bf, xsqv[c][:, s, :],
                                             start=(c == 0 and s == 0),
                                             stop=(c == 3 and s == 1))
                    nc.scalar.activation(nrm_t[:], ss_ps[:], Act.Sqrt, scale=256.0)
                    if USE_RECIP_APPROX:
                        nc.vector.reciprocal_approx_fast(out=inv_t[:], in_=nrm_t[:])
                    else:
                        nc.vector.reciprocal(inv_t[:], nrm_t[:])
                    # preload exp table behind the DMA/matmul wait
                    nc.scalar.activation(sgs_t[0:1, 0:1], bp_t[0:1, 256:257], Act.Exp)

                    def pair(e2):
                        yps = [ps1.tile([128, M], f32, name=f"y{e2}_{s}", tag="mm",
                                        bufs=4) for s in range(2)]
                        for c in range(4):
                            for s in range(2):
                                nc.tensor.matmul(
                                    yps[s][:], w1sv[e2][:, c, :, s * 128:(s + 1) * 128],
                                    xpv[c], start=(c == 0), stop=(c == 3),
                                    perf_mode=DR)
                        for s in range(2):
                            nc.vector.scalar_tensor_tensor(
                                out=ybv[e2][:, s, :], in0=yps[s][:], scalar=32.0,
                                in1=inv_t[:], op0=Alu.mult, op1=Alu.mult)

                    def transposes(e2):
                        # 8 fp8 transposes (2 e-tiles x 4 m) packed in ONE 2KB bank
                        # (fp8 transpose requires output element step of 2)
                        tp = ps1.tile([128, 2048], fp8, name=f"t{e2}",
                                      tag="mm", bufs=4)
                        tpv = tp.rearrange("p (k e two) -> p k e two", k=8, two=2)
                        for m in range(MT):
                            for s in range(2):
                                k = m * 2 + s
                                nc.tensor.transpose(
                                    tpv[:, k, :, 0],
                                    ybv[e2][:, s, m * 128:(m + 1) * 128], cf8_t)
                        for m in range(MT):
                            # scalar engine (idle in phase 1) does the psum->bf16
                            # copies; DVE stays free for ybp STTs
                            nc.scalar.activation(
                                yT[m][:, e2 * 256:(e2 + 1) * 256].rearrange(
                                    "p (s e) -> p s e", s=2),
                                tpv[:, 2 * m:2 * m + 2, :, 0], Act.Copy)

                    # emission: pair0, pair1, gate, then lag-1 T/lg with pairs
                    pair(0)
                    pair(1)
                    gate_ps = ps1.tile([G, M], f32, name="gate", tag="ssg", bufs=1)
                    for c in range(4):
                        nc.tensor.matmul(gate_ps[:], wgv[:, c, :, 0:G], xpv[c],
                                         start=(c == 0), stop=(c == 3), perf_mode=DR)
                    nc.vector.tensor_mul(sgs_t[:], gate_ps[:], inv_t[0:G, :])
                    nc.scalar.activation(sgs_t[:], sgs_t[:], Act.Tanh,
                                         bias=bp_t[:, 258:259], scale=0.5)
                    nc.vector.tensor_scalar(out=sg_t[:], in0=sgs_t[:], scalar1=0.5,
                                            scalar2=0.5, op0=Alu.mult, op1=Alu.add)
                    lg_tile(0)
                    lg_tile(1)
                    for e2 in range(NE2):
                        if e2 + 2 < NE2:
                            pair(e2 + 2)
                        transposes(e2)
                        lg_chain(0, e2)
                        lg_chain(1, e2)

                # ---------------- phase 2/3 ----------------
                with tc.tile_pool(name="ps3", bufs=1, space="PSUM") as ps3:
                    vd_ps = ps3.tile([128, 512], f32, name="vd", tag="vd", bufs=1)

                    # broadcast sg rows to 128 partitions via one-hot matmul
                    for g in range(G):
                        sgb_ps = ps3.tile([128, M], f32, name=f"sgb{g}", tag="p3",
                                          bufs=3)
                        nc.tensor.matmul(
                            sgb_ps[:], onehot[0:G, g * 128:(g + 1) * 128],
                            sg_t[:], start=True, stop=True)
                        nc.vector.tensor_copy(sgb[g][:], sgb_ps[:])

                    def softwf(g):
                        if USE_ACCUM_OUT:
                            nc.scalar.activation(ex_t[g][:], lg_ps[g][:], Act.Exp,
                                                 scale=1.0 / 256.0,
                                                 accum_out=se_t[:, g:g + 1])
                        else:
                            nc.scalar.activation(ex_t[g][:], lg_ps[g][:], Act.Exp,
                                                 scale=1.0 / 256.0)
                            nc.vector.reduce_sum(out=se_t[:, g:g + 1],
                                                 in_=ex_t[g][:],
                                                 axis=mybir.AxisListType.X)
                        nc.vector.reciprocal(ise_t[:, g:g + 1], se_t[:, g:g + 1])
                        if USE_ACCUM_OUT:
                            nc.vector.scalar_tensor_tensor(
                                out=wf_t[g][:], in0=ex_t[g][:],
                                scalar=ise_t[:, g:g + 1],
                                in1=sgb[g][:], op0=Alu.mult, op1=Alu.mult,
                                accum_out=sgs_acc[:, g:g + 1])
                        else:
                            nc.vector.scalar_tensor_tensor(
                                out=wf_t[g][:], in0=ex_t[g][:],
                                scalar=ise_t[:, g:g + 1],
                                in1=sgb[g][:], op0=Alu.mult, op1=Alu.mult)
                            nc.vector.reduce_sum(out=sgs_acc[:, g:g + 1],
                                                 in_=wf_t[g][:],
                                                 axis=mybir.AxisListType.X)

                    def wfT_einsum(g):
                        tp = ps3.tile([128, 1024], bf16, name=f"wt{g}", tag="p3",
                                      bufs=3)
                        for m in range(MT):
                            nc.tensor.transpose(tp[:, m * 128:(m + 1) * 128],
                                                wf_t[g][:, m * 128:(m + 1) * 128],
                                                identbf)
                        nc.vector.tensor_copy(wfT_t[g][:], tp[:, 0:M])
                        for m in range(MT):
                            nc.tensor.matmul(
                                vd_ps[:, 0:256],
                                wfT_t[g][:, m * 128:(m + 1) * 128],
                                yT[m][:, g * 256:(g + 1) * 256],
                                start=(g == 0 and m == 0), stop=False)

                    for gp in range(1, 4):
                        g0, g1 = 2 * gp, 2 * gp + 1
                        pv0, pv1 = 2 * (gp - 1), 2 * (gp - 1) + 1
                        softwf(pv0)
                        softwf(pv1)
                        lg_tile(g0)
                        for e2 in range(NE2):
                            lg_chain(g0, e2)
                        wfT_einsum(pv0)
                        lg_tile(g1)
                        for e2 in range(NE2):
                            lg_chain(g1, e2)
                        wfT_einsum(pv1)
                    for g in (6, 7):
                        softwf(g)
                        wfT_einsum(g)
                    # swap back to sqrt table behind the einsum tail
                    nc.scalar.activation(nrm_t[0:1, 0:1], bp_t[0:1, 256:257], Act.Sqrt)

                    # correction: SgT = transpose(Sg); vd += SgT.T @ [32*binp | 32]
                    sgT_ps = ps3.tile([128, 512], f32, name="sgT", tag="p3", bufs=3)
                    nc.tensor.transpose(sgT_ps[0:G, 0:128], sgs_acc[:], identf32)
                    nc.vector.tensor_copy(sgT_sb[:], sgT_ps[0:G, 0:128])
                    nc.tensor.matmul(vd_ps[:, 0:256], sgT_sb[:], bp_t[:, 0:256],
                                     start=False, stop=True)
                    nc.tensor.matmul(vd_ps[:, 256:257], sgT_sb[:], bp_t[:, 256:257],
                                     start=True, stop=True)

                    nc.vector.scalar_tensor_tensor(
                        out=vlad_t[:], in0=centn_t[:], scalar=vd_ps[:, 256:257],
                        in1=vd_ps[:, 0:256], op0=Alu.mult, op1=Alu.add)
                    if USE_TTR:
                        nc.vector.tensor_tensor_reduce(
                            out=sq_t[:], in0=vlad_t[:], in1=vlad_t[:], scale=1.0,
                            scalar=0.0, op0=Alu.mult, op1=Alu.add,
                            accum_out=ss2_t[:])
                    else:
                        nc.vector.tensor_mul(sq_t[:], vlad_t[:], vlad_t[:])
                        nc.vector.reduce_sum(out=ss2_t[:], in_=sq_t[:],
                                             axis=mybir.AxisListType.X)
                    nc.scalar.activation(nr2_t[:], ss2_t[:], Act.Sqrt, scale=128.0)
                    nc.vector.reciprocal(r1_t[:], nr2_t[:])
                    nc.vector.tensor_scalar_mul(out_t[:], vlad_t[:], r1_t[:])
                    nc.sync.dma_start(out=out_d[:], in_=out_t[:])

    nc.compile()
    return nc


def _get_nc():
    if "nc" not in _cache:
        _cache["nc"] = _build_nc()
    return _cache["nc"]


def host_inputs(x, W_inp, b_inp, W_g, b_g, W_gk, b_gk, centroids):
    import ml_dtypes as mld

    x = np.asarray(x, dtype=np.float32)
    X = x.reshape(8, 8, N, 64).transpose(0, 2, 1, 3).reshape(8, N, M)
    WgT = ((np.asarray(W_g, np.float64) @ np.asarray(W_inp, np.float64)).T
           ).astype(np.float32)
    W1 = np.ascontiguousarray(
        (np.concatenate([np.asarray(W_inp, np.float32).T, WgT],
                        axis=1) * 16.0).astype(mld.float8_e4m3))
    W2 = np.ascontiguousarray(
        (np.asarray(W_gk, np.float32).T * 8.0).astype(mld.float8_e4m3))
    bg = (np.asarray(b_g, np.float64)
          + np.asarray(W_g, np.float64) @ np.asarray(b_inp, np.float64)
          ).astype(np.float32)
    oh = np.zeros((128, 1024), np.float32)
    for g in range(G):
        oh[g, g * 128:(g + 1) * 128] = 1.0
    cbf = np.concatenate([np.ones((128, 128), np.float32),
                          np.eye(128, dtype=np.float32), oh],
                         axis=1).astype(mld.bfloat16)
    cf8 = np.eye(128, dtype=np.float32).astype(mld.float8_e4m3)
    cf32 = np.concatenate([-np.asarray(centroids, np.float32),
                           np.eye(128, dtype=np.float32)], axis=1)
    cf32 = np.ascontiguousarray(cf32)
    bp = np.zeros((G, 260), np.float32)
    bp[:, 0:256] = np.asarray(b_inp, np.float32).reshape(G, D) * 32.0
    bp[:, 256] = 32.0
    bp[:, 258] = 0.5 * bg
    common = {"w1": W1, "w2": W2, "cbf": np.ascontiguousarray(cbf),
              "cf8": np.ascontiguousarray(cf8), "cf32": cf32, "bp": bp}
    in_maps = []
    for b in range(8):
        m = dict(common)
        m["x"] = np.ascontiguousarray((X[b] * 8.0).astype(mld.float8_e4m3))
        in_maps.append(m)
    return in_maps


def kernel(x, W_inp, b_inp, W_g, b_g, W_gk, b_gk, centroids):
    from concourse.bass_utils import run_bass_kernel_spmd

    nc = _get_nc()
    in_maps = host_inputs(x, W_inp, b_inp, W_g, b_g, W_gk, b_gk, centroids)
    trace = os.environ.get("KERNEL_TRACE") == "1"
    r = run_bass_kernel_spmd(nc, in_maps, core_ids=list(range(8)), trace=trace)
    _cache["last_results"] = r
    return np.stack([r.results[b]["out"].reshape(KC * D)
                     for b in range(8)]).astype(np.float32)
